# revision 47
# baseline (speedup 1.0000x reference)
"""Trainium2 Bass kernel for per-gene linear layer.

Math (reference):
    gene    = x[:, :20000]           # (B, G)
    nongene = x[:, 20000:]           # (B, K=128)
    y[:, g] = gene[:, g] * W[g, 0] + nongene @ W[g, 1:] + b[g]

Sharding: model parallel over genes across 8 cores (2500 genes each,
padded to 2560 = 20 tiles of 128 for uniform SPMD tiling).

The per-gene diagonal weight dw is folded into the gene block on the
host (xgs = gene * dw, fp8), so the diagonal term is a plain add:

  DVE tiles (odd):  out = bf16((xgs + b) + psum)  one scalar_tensor_tensor
  ACT tiles (even): psum += I.T @ xgs   (TensorE, fp8 identity lhsT)
                    out = bf16(psum + b)          one activation pass

This splits the epilogue evenly across both elementwise engines with
zero extra passes and cuts TensorE work ~25% vs diag-matmul-everywhere.

Traffic (per core, the binding roofline for this memory-regime op):
    loads  xgs 2.62 MB fp8 + wsh 0.66 MB bf16 + xn 0.26 MB bf16
    stores y 5.24 MB bf16 (upcast to f32 on host during unshard)

Perf notes: every HWDGE dma_start costs ~600ns of sequencer time and
each DMA-completion semaphore fires ~1.5-3us after the data lands
(HBM write-receipt round trip), so DMAs are consolidated into a
handful of large transfers and the critical head chunks (xn, wsh head,
xg head) are ordered first on their rings. Three descriptor streams
run concurrently (SP HWDGE = Sync, ACT HWDGE = ScalarE, SWDGE =
GpSimd Q7); read+write mixes sustain ~400 GB/s vs ~280 for a single
ring. TensorE gets a ~4us dummy-matmul warmup so the HAM clock gate
is released (1.2 -> 2.4 GHz) before the first real matmul, and the
load schedule keeps any TensorE stall under ~2us so the gate never
re-throttles mid-run.
"""

import os
import numpy as np
from contextlib import ExitStack

import concourse.bass as bass
import concourse.tile as tile
from concourse import bacc, mybir
from concourse.bass_utils import run_bass_kernel_spmd

B = 1024           # batch
G = 20000          # genes (output dim)
K = 128            # shared nongene features
IN_DIM = G + K     # 20128
N_CORES = 8
G_CORE = G // N_CORES            # 2500 genes per core
N_GT = 20                        # gene tiles per core (padded)
G_PAD = N_GT * 128               # 2560

WSH_CHUNKS = [(0, 4), (4, 20)]
# xg chunks: the head chunk rides the ACT ring (lands first, gates the
# first epilogues), the bulk rides the SP ring behind the weights. All
# loads land by ~16us: a late xg chunk stalls TensorE >2.5us and
# re-throttles the HAM clock gate to 1.2 GHz.
XG_CHUNKS_SP = [(4, 8), (8, 12), (12, 16), (16, 20)]
XG_CHUNKS_ACT_HEAD = [(0, 4)]
# store chunking: (start_tile, end_tile, ring), spread over all three
# streams so the output drains at the combined HBM write rate. The tail
# chunks are 1-2 tiles on independent streams so the last few epilogues
# (which alternate ScalarE/DVE) each trigger their store immediately.
ST_CHUNKS = [(0, 2, "gp"), (2, 5, "sp"), (5, 8, "gp"), (8, 11, "gp"),
             (11, 14, "act"), (14, 18, "sp"), (18, 20, "act")]
# tiles whose epilogue runs on ScalarE (these also get the diag matmul;
# the rest fold the diag into the DVE scalar_tensor_tensor epilogue).
# Tile 0 is ScalarE so both elementwise engines start streaming output
# as soon as the first xg chunk's completion semaphore fires.
SCALAR_TILES = {0, 2, 4, 6, 8, 10, 12, 14, 16, 18}

_NC_CACHE = None
LAST_RESULTS = None  # BassKernelResults of the most recent run (for test harness)


def _build_nc():
    nc = bacc.Bacc("TRN2", target_bir_lowering=False, debug=False,
                   enable_asserts=True, num_devices=N_CORES)
    f32 = mybir.dt.float32
    bf16 = mybir.dt.bfloat16
    fp8 = mybir.dt.float8e4

    xg_d = nc.dram_tensor("xg", [128, N_GT * B], fp8, kind="ExternalInput").ap()
    wshT = nc.dram_tensor("wshT", [K, G_PAD], bf16, kind="ExternalInput").ap()
    xnT = nc.dram_tensor("xnT", [K, B], bf16, kind="ExternalInput").ap()
    id_d = nc.dram_tensor("idm", [128, 128], fp8, kind="ExternalInput").ap()
    bt = nc.dram_tensor("bt", [128, N_GT], f32, kind="ExternalInput").ap()
    y_d = nc.dram_tensor("y", [128, N_GT * B], bf16, kind="ExternalOutput").ap()

    with tile.TileContext(nc) as tc, ExitStack() as ctx:
        const = ctx.enter_context(tc.tile_pool(name="const", bufs=1))
        # one buffer per store chunk: an out tile must never wait on an
        # earlier chunk's store completion (SWDGE completions are slow)
        out_pool = ctx.enter_context(
            tc.tile_pool(name="out", bufs=len(ST_CHUNKS)))
        psum_pool = ctx.enter_context(
            tc.tile_pool(name="psum", bufs=4, space="PSUM"))

        # ---- head DMAs -------------------------------------------------
        # SP ring: xn first (gates first matmul), then the wsh bulk and
        # the xg bulk so epilogues unblock progressively.
        xn_s = const.tile([K, B], bf16)
        nc.sync.dma_start(xn_s[:, :512], xnT[:, :512])
        nc.sync.dma_start(xn_s[:, 512:], xnT[:, 512:])
        wsh_s = const.tile([K, G_PAD], bf16)
        xg_s = const.tile([128, N_GT * B], fp8)
        s0, e0 = WSH_CHUNKS[0]
        nc.sync.dma_start(wsh_s[:, s0 * 128:e0 * 128],
                          wshT[:, s0 * 128:e0 * 128])
        s1, e1 = WSH_CHUNKS[1]
        nc.sync.dma_start(wsh_s[:, s1 * 128:e1 * 128],
                          wshT[:, s1 * 128:e1 * 128])
        for sx, ex in XG_CHUNKS_SP:
            nc.sync.dma_start(xg_s[:, sx * B:ex * B], xg_d[:, sx * B:ex * B])

        # ACT ring: the head xg chunk first (gates the first epilogues),
        # then output stores later.
        for sx, ex in XG_CHUNKS_ACT_HEAD:
            nc.scalar.dma_start(xg_s[:, sx * B:ex * B], xg_d[:, sx * B:ex * B])

        # GpSimd SWDGE: the tiny identity + bias loads (Q7 emission is
        # slow but these are off the critical trigger queues) and two
        # middle store chunks.
        id_s = const.tile([128, 128], fp8)
        nc.gpsimd.dma_start(id_s[:], id_d[:])
        b_s = const.tile([128, N_GT], f32)
        nc.gpsimd.dma_start(b_s[:], bt[:])

        # warm the ACT function table during the DMA head so the first
        # real ACTIVATE doesn't eat the ~2.7us table load.
        warm_src = const.tile([128, 512], bf16)
        nc.vector.memset(warm_src[:], 0.0)
        warm = const.tile([128, 1], f32)
        nc.scalar.activation(warm[:], warm_src[:, 0:1],
                             mybir.ActivationFunctionType.Identity,
                             bias=0.0, scale=1.0)

        # ~3.4us of dummy matmuls so the PE HAM clock-gate is released
        # (1.2 -> 2.4 GHz) before the first real matmul. Runs while the
        # head DMAs are still in flight; reuses psum slot 0 (the pool
        # serializes real tile 3 behind it, which is fine - these finish
        # long before).
        warm_psum = psum_pool.tile([128, B], f32, tag="ps")
        for _ in range(11):
            nc.tensor.matmul(warm_psum[:, :512], warm_src[:, :128],
                             warm_src[:, :512], start=True, stop=True)

        def mm_wsh(psum, gt, close):
            wl = wsh_s[:, gt * 128:(gt + 1) * 128]
            for h in range(2):
                c0 = h * 512
                nc.tensor.matmul(psum[:, c0:c0 + 512], wl,
                                 xn_s[:, c0:c0 + 512],
                                 start=True, stop=close)

        def mm_diag(psum, gt):
            for h in range(2):
                c0 = h * 512
                nc.tensor.matmul(psum[:, c0:c0 + 512], id_s[:],
                                 xg_s[:, gt * B + c0:gt * B + c0 + 512],
                                 start=False, stop=True)

        # ---- main pipeline --------------------------------------------
        chunk_idx = 0
        out_sup = None
        out_base = 0
        for t in range(N_GT):
            psum = psum_pool.tile([128, B], f32, tag="ps")
            cs, ce, ring = ST_CHUNKS[chunk_idx]
            if out_sup is None:
                out_sup = out_pool.tile([128, (ce - cs) * B], bf16)
                out_base = cs
            ob = out_sup[:, (t - out_base) * B:(t - out_base + 1) * B]
            bias = b_s[:, t:t + 1]
            if t in SCALAR_TILES:
                mm_wsh(psum, t, close=False)
                mm_diag(psum, t)
                nc.scalar.activation(ob, psum[:],
                                     mybir.ActivationFunctionType.Identity,
                                     bias=bias, scale=1.0)
            else:
                mm_wsh(psum, t, close=True)
                # out = (xgs + b) + psum : diag term + bias + matmul in
                # one DVE pass
                nc.vector.scalar_tensor_tensor(
                    ob, xg_s[:, t * B:(t + 1) * B], bias, psum[:],
                    op0=mybir.AluOpType.add, op1=mybir.AluOpType.add)
            if t == ce - 1:
                dst = y_d[:, cs * B:ce * B]
                if ring == "act":
                    nc.scalar.dma_start(dst, out_sup[:])
                elif ring == "gp":
                    nc.gpsimd.dma_start(dst, out_sup[:])
                else:
                    nc.sync.dma_start(dst, out_sup[:])
                out_sup = None
                chunk_idx += 1

    nc.compile()
    return nc


def _get_nc():
    global _NC_CACHE
    if _NC_CACHE is None:
        _NC_CACHE = _build_nc()
    return _NC_CACHE


def kernel(x, W, b):
    global LAST_RESULTS
    import ml_dtypes
    x = np.asarray(x, dtype=np.float32)
    W = np.asarray(W, dtype=np.float32)
    b = np.asarray(b, dtype=np.float32)
    assert x.shape == (B, IN_DIM) and W.shape == (G, 1 + K) and b.shape == (G,)

    xT = np.ascontiguousarray(x.T)          # (20128, 1024)
    xnT = xT[G:].astype(ml_dtypes.bfloat16)  # (128, 1024), replicated
    # gene block pre-scaled by the per-gene diagonal weight, fp8, packed
    # partition-major per core: [core, p, j, e] with gene = j*128 + p
    xg_pad = np.zeros((N_CORES, G_PAD, B), ml_dtypes.float8_e4m3)
    xg_pad[:, :G_CORE] = (xT[:G] * W[:, 0:1]).astype(
        ml_dtypes.float8_e4m3).reshape(N_CORES, G_CORE, B)
    xg_pm = np.ascontiguousarray(
        xg_pad.reshape(N_CORES, N_GT, 128, B).transpose(0, 2, 1, 3)).reshape(
        N_CORES, 128, N_GT * B)

    ident = np.eye(128, dtype=ml_dtypes.float8_e4m3)

    in_maps = []
    for c in range(N_CORES):
        g0 = c * G_CORE
        Wc = W[g0:g0 + G_CORE]

        def cols(v):
            m = np.zeros((128, N_GT), np.float32)
            m[:, :G_CORE // 128] = v[:(G_CORE // 128) * 128].reshape(-1, 128).T
            rem = G_CORE - (G_CORE // 128) * 128
            if rem:
                m[:rem, G_CORE // 128] = v[(G_CORE // 128) * 128:]
            return m

        wsh = np.zeros((K, G_PAD), ml_dtypes.bfloat16)
        wsh[:, :G_CORE] = Wc[:, 1:].T.astype(ml_dtypes.bfloat16)
        in_maps.append({
            "xg": xg_pm[c],
            "wshT": wsh,
            "xnT": xnT,
            "idm": ident,
            "bt": cols(np.ascontiguousarray(b[g0:g0 + G_CORE])),
        })

    nc = _get_nc()
    trace = bool(os.environ.get("KERNEL_TRACE"))
    kwargs = {}
    if trace:
        tdir = os.environ.get("KERNEL_TRACE_DIR")
        if tdir:
            os.makedirs(tdir, exist_ok=True)
            kwargs["tmpdir"] = tdir
    LAST_RESULTS = run_bass_kernel_spmd(nc, in_maps, list(range(N_CORES)),
                                        trace=trace, **kwargs)
    y = np.empty((B, G), np.float32)
    yT_view = y.T  # fill transposed view to avoid a second big copy
    for c in range(N_CORES):
        # device layout [p, j, e] -> gene-major [j*128+p, e], upcast bf16->f32
        yc = LAST_RESULTS.results[c]["y"].reshape(128, N_GT, B)
        yT_view[c * G_CORE:(c + 1) * G_CORE] = \
            yc.transpose(1, 0, 2).reshape(G_PAD, B)[:G_CORE].astype(np.float32)
    return y
